# revision 14
# baseline (speedup 1.0000x reference)
"""Trainium2 Bass kernel for nn_LossWithBeliveMaps.

loss = mean((prediction - belive_map)^2) where belive_map is 100 Gaussian
(9x9, sigma=2) stamps per image, scattered at integer keypoint coordinates.

Decomposition (per image):  loss*N = S1 - 2*S2 + S3 with
  S1 = sum(pred^2)            -- streamed square+accumulate, no dependency
                                 on keypoints, starts as soon as DMA lands.
  S2 = sum(pred * bm)         -- bm = Ay^T Bx is rank-100 separable
                                 (G[i,j] = u(i)u(j), u(d) = exp(-d^2/8)), so
                                 S2 = sum_k w_k sum_c U[k,c] Bx[k,c] with
                                 U = Ayt^T @ pred contracted on the PE per
                                 128-row block (pred streams as float32r at
                                 full rate), then one tiny [100,1024] fused
                                 multiply-reduce on DVE.
  S3 = sum(bm^2)              -- = sum_{k,k'} w_k w_k' gy[k,k'] gx[k,k']
                                 via [100,100] Gram matmuls of the factors.
  w_k in {0,1} removes duplicate keypoints (.at[].set semantics); weights
  are applied on the small [100,*] tensors only.

The 9x9 hard cutoff of the reference kernel is approximated by the full
Gaussian tails (exp(-25/8) ~ 0.04 max excess, shifting S2/S3 by ~0.2%,
i.e. ~3e-6 relative on the loss -- tolerance is 2e-2).

Sharding: data-parallel over batch, 2 images per core, 8 cores; host sums
per-core partial columns in float64.
"""

import numpy as np

import concourse.bass as bass
import concourse.bacc as bacc
import concourse.bass_isa as bass_isa
import concourse.mybir as mybir
from concourse import tile
from concourse.bass_utils import run_bass_kernel_spmd

F32 = mybir.dt.float32
F32R = mybir.dt.float32r
I32 = mybir.dt.int32
BF16 = mybir.dt.bfloat16
OP = mybir.AluOpType
AF = mybir.ActivationFunctionType

B, H, W = 16, 1024, 1024
NKP = 100
NCORES = 8
IMGS = B // NCORES            # 2 images per core
NT = H // 128                 # 8 row-blocks per image
RB = 2                        # row-blocks per pred tile
NTL = NT // RB                # 4 pred tiles per image
# acc columns: [0..7] S1 per tile, [8..9] S2 per image, [10..11] S3
NCOL = IMGS * NTL + 2 * IMGS


def build_nc():
    nc = bacc.Bacc(None, target_bir_lowering=False)

    pred = nc.dram_tensor("pred", [IMGS, H, W], F32R, kind="ExternalInput")
    coords = nc.dram_tensor("coords", [IMGS, NKP, 2], I32, kind="ExternalInput")
    out = nc.dram_tensor("partial", [128, NCOL], F32, kind="ExternalOutput")

    with tile.TileContext(nc) as tc:
        with (
            tc.tile_pool(name="const", bufs=1) as constp,
            tc.tile_pool(name="fact", bufs=1) as factp,
            tc.tile_pool(name="pred", bufs=IMGS * (NT // RB)) as predp,
            tc.tile_pool(name="work", bufs=2) as workp,
            tc.tile_pool(name="small", bufs=2) as smallp,
            tc.tile_pool(name="acc", bufs=1) as accp,
            tc.tile_pool(name="psum", bufs=1, space="PSUM") as psump,
        ):
            acc = accp.tile([128, NCOL], F32)
            nc.gpsimd.memset(acc[:], 0)

            # ---- constants, built on-chip (no DMA bandwidth spent) ----
            iota_i = constp.tile([128, W], I32)
            nc.gpsimd.iota(iota_i[:], [[1, W]], channel_multiplier=0)
            iota_f = constp.tile([128, W], F32)
            nc.vector.tensor_copy(iota_f[:], iota_i[:])

            iotap_i = constp.tile([128, 1], I32)
            nc.gpsimd.iota(iotap_i[:], [[1, 1]], channel_multiplier=1)
            iotap_f = constp.tile([128, 1], F32)
            nc.vector.tensor_copy(iotap_f[:], iotap_i[:])

            # rconst[p, a] = p + 128*a  (row index; broadcast over k)
            rconst_f = constp.tile([128, NT], F32)
            nc.vector.tensor_scalar(rconst_f[:], iota_f[:, 0:NT], 128.0,
                                    iotap_f[:], OP.mult, OP.add)
            ones_col = constp.tile([NKP, 1], F32)
            nc.gpsimd.memset(ones_col[:], 1.0)

            # mask_lt[k, k'] = (k' < k), mask_gt[k, k'] = (k' > k)
            mask_lt = constp.tile([NKP, NKP], F32)
            nc.vector.tensor_scalar(mask_lt[:], iota_f[0:NKP, 0:NKP],
                                    iotap_f[0:NKP], None, OP.is_lt)
            mask_gt = constp.tile([NKP, NKP], F32)
            nc.vector.tensor_scalar(mask_gt[:], iota_f[0:NKP, 0:NKP],
                                    iotap_f[0:NKP], None, OP.is_gt)

            # ---- prediction layout: [128, RB, W] tiles ----
            pred_v = pred.rearrange("i (t b p) w -> i t p b w", b=RB, p=128)
            pts = {}

            def load_pt(img, t):
                pt = predp.tile([128, RB, W], F32R, tag="pt",
                                name=f"pt{img}_{t}")
                nc.sync.dma_start(pt[:], pred_v[img, t])
                pts[(img, t)] = pt

            # ---- coordinate loads first: they unlock the longest chains
            ccs, crxs, crys = [], [], []
            for img in range(IMGS):
                cc = smallp.tile([NKP, 2], I32, tag=f"cc{img}", bufs=1)
                nc.sync.dma_start(cc[:], coords[img])
                ctv = coords[img].rearrange("n t -> t n")
                crx = smallp.tile([1, NKP], I32, tag=f"crx{img}", bufs=1)
                nc.sync.dma_start(crx[:], ctv[0:1, :])
                cry = smallp.tile([1, NKP], I32, tag=f"cry{img}", bufs=1)
                nc.sync.dma_start(cry[:], ctv[1:2, :])
                ccs.append(cc); crxs.append(crx); crys.append(cry)

            for img in range(IMGS):
                for t in range(NTL):
                    load_pt(img, t)

            # ---- tiny coordinate conversions for both images first ----
            ayts, bxts, bx0s = [], [], []
            xbs, ybs, ccfs = [], [], []
            for img in range(IMGS):
                ccf = smallp.tile([NKP, 2], F32, tag="ccf", bufs=1,
                                  name=f"ccf{img}")
                nc.vector.tensor_copy(ccf[:], ccs[img][:])
                crxf = smallp.tile([1, NKP], F32, tag="crxf", bufs=2,
                                   name=f"crxf{img}")
                nc.vector.tensor_copy(crxf[:], crxs[img][:])
                cryf = smallp.tile([1, NKP], F32, tag="cryf", bufs=2,
                                   name=f"cryf{img}")
                nc.vector.tensor_copy(cryf[:], crys[img][:])
                xb = smallp.tile([128, NKP], F32, tag=f"xb{img}", bufs=1)
                nc.gpsimd.partition_broadcast(xb[:], crxf[:])
                yb = smallp.tile([128, NKP], F32, tag=f"yb{img}", bufs=1)
                nc.gpsimd.partition_broadcast(yb[:], cryf[:])
                ccfs.append(ccf); xbs.append(xb); ybs.append(yb)

            # ---- factor chains: d (DVE) -> d^2 (GpSimd) -> exp (ACT) ----
            for img in range(IMGS):
                facs = []
                for bvec, dtag in ((ybs[img], "dy"), (xbs[img], "dx")):
                    bexp = bvec[:].unsqueeze(1).broadcast_to([128, NT, NKP])
                    d = workp.tile([128, NT, NKP], F32, tag="d")
                    rexp = rconst_f[:].unsqueeze(2).broadcast_to(
                        [128, NT, NKP])
                    nc.vector.tensor_tensor(d[:], rexp, bexp, OP.subtract)
                    dsq = workp.tile([128, NT, NKP], F32, tag="dsq")
                    nc.gpsimd.tensor_tensor(dsq[:], d[:], d[:], OP.mult)
                    f = factp.tile([128, NT, NKP], F32R, tag=f"{dtag}{img}")
                    nc.scalar.activation(f[:], dsq[:], AF.Exp, scale=-0.125)
                    facs.append(f)
                ayt, bxt = facs
                ayts.append(ayt); bxts.append(bxt)

                bd = workp.tile([NKP, W], F32, tag="bd")
                nc.vector.tensor_scalar(bd[:], iota_f[0:NKP, :],
                                        ccfs[img][:, 0:1], None, OP.subtract)
                bdsq = workp.tile([NKP, W], F32, tag="bdsq")
                nc.vector.tensor_tensor(bdsq[:], bd[:], bd[:], OP.mult)
                bx0 = factp.tile([NKP, W], F32, tag=f"bx0_{img}")
                nc.scalar.activation(bx0[:], bdsq[:], AF.Exp, scale=-0.125)
                bx0s.append(bx0)

            # ---- dedup weights (off the factor critical path) ----
            wcols, walls = [], []
            for img in range(IMGS):
                idb = smallp.tile([NKP, NKP], F32, tag="idb")
                nc.vector.tensor_scalar(idb[:], ybs[img][0:NKP, :], 1024.0,
                                        None, OP.mult)
                nc.vector.tensor_tensor(idb[:], idb[:], xbs[img][0:NKP, :],
                                        OP.add)
                idc = smallp.tile([NKP, 1], F32, tag="idc")
                nc.vector.tensor_scalar(idc[:], ccfs[img][:, 1:2], 1024.0,
                                        ccfs[img][:, 0:1], OP.mult, OP.add)
                eq = smallp.tile([NKP, NKP], F32, tag="eq")
                nc.vector.tensor_scalar(eq[:], idb[:], idc[:], None,
                                        OP.is_equal)
                e1 = smallp.tile([NKP, NKP], F32, tag="e1")
                nc.vector.tensor_tensor(e1[:], eq[:], mask_lt[:], OP.mult)
                dup = smallp.tile([NKP, 1], F32, tag="dup")
                nc.vector.tensor_reduce(dup[:], e1[:],
                                        axis=mybir.AxisListType.X, op=OP.add)
                w_col = smallp.tile([NKP, 1], F32, tag=f"wcol{img}", bufs=1)
                nc.vector.tensor_scalar(w_col[:], dup[:], 0.0, None, OP.is_le)
                e2 = smallp.tile([NKP, NKP], F32, tag="e2")
                nc.vector.tensor_tensor(e2[:], eq[:], mask_gt[:], OP.mult)
                cntr = psump.tile([1, NKP], F32, tag=f"cntr{img}",
                                  name=f"cntr{img}")
                nc.tensor.matmul(cntr[:], ones_col[:], e2[:],
                                 start=True, stop=True)
                wrow = smallp.tile([1, NKP], F32, tag="wrow")
                nc.vector.tensor_scalar(wrow[:], cntr[:], 0.0, None, OP.is_le)
                w_all = smallp.tile([NKP, NKP], F32, tag=f"wall{img}", bufs=1)
                nc.gpsimd.partition_broadcast(w_all[:], wrow[:])
                wcols.append(w_col); walls.append(w_all)

            # ---- PSUM: U [100, 1024] per image + grams packed in 1 bank
            us = [psump.tile([NKP, W], F32, tag=f"u{img}", name=f"u{img}")
                  for img in range(IMGS)]
            ggs = [psump.tile([NKP, 256], F32, tag=f"gg{img}", name=f"gg{img}")
                   for img in range(IMGS)]

            # ---- Gram matmuls (bf16, tiny): gy = Ayt^T Ayt, gx = Bxt^T Bxt
            for img in range(IMGS):
                for a in range(NT):
                    ay = ayts[img][:, a, :]
                    nc.tensor.matmul(ggs[img][:, 0:NKP], ay, ay,
                                     start=(a == 0), stop=(a == NT - 1))
                for a in range(NT):
                    bx = bxts[img][:, a, :]
                    nc.tensor.matmul(ggs[img][:, 128:128 + NKP], bx, bx,
                                     start=(a == 0), stop=(a == NT - 1))

            # ---- main stream: S1 square+accum and U accumulation ----
            for img in range(IMGS):
                for t in range(NTL):
                    pt = pts[(img, t)]
                    col = img * NTL + t
                    junk = workp.tile([128, RB, W], BF16, tag="junk_act")
                    nc.scalar.activation(junk[:], pt[:].bitcast(F32),
                                         AF.Square,
                                         accum_out=acc[:, col:col + 1])
                    for b in range(RB):
                        for s in range(2):
                            nc.tensor.matmul(
                                us[img][:, s * 512:(s + 1) * 512],
                                ayts[img][:, RB * t + b, :],
                                pt[:, b, s * 512:(s + 1) * 512],
                                start=(t == 0 and b == 0),
                                stop=(t == NTL - 1 and b == RB - 1))

                # -- close out image: S2 and S3 reductions
                s2c = smallp.tile([NKP, 1], F32, tag="s2c")
                junk2 = workp.tile([NKP, W], F32, tag="junk2")
                nc.vector.tensor_tensor(junk2[:], us[img][:], bx0s[img][:],
                                        OP.mult)
                nc.vector.tensor_reduce(s2c[:], junk2[:],
                                        axis=mybir.AxisListType.X, op=OP.add)
                nc.vector.tensor_tensor(acc[0:NKP, IMGS * NTL + img:IMGS * NTL + img + 1],
                                        s2c[:], wcols[img][:], OP.mult)

                t1 = smallp.tile([NKP, NKP], F32, tag="t1")
                nc.vector.tensor_tensor(t1[:], ggs[img][:, 0:NKP],
                                        walls[img][:], OP.mult)
                s3c = smallp.tile([NKP, 1], F32, tag="s3c")
                junk3 = smallp.tile([NKP, NKP], F32, tag="junk3")
                nc.vector.tensor_tensor(junk3[:], t1[:],
                                        ggs[img][:, 128:128 + NKP], OP.mult)
                nc.vector.tensor_reduce(s3c[:], junk3[:],
                                        axis=mybir.AxisListType.X, op=OP.add)
                nc.vector.tensor_tensor(acc[0:NKP, IMGS * NTL + IMGS + img:IMGS * NTL + IMGS + img + 1],
                                        s3c[:], wcols[img][:], OP.mult)

            nc.sync.dma_start(out[:], acc[:])

    nc.compile()
    return nc


_NC_CACHE = {}


def _get_nc():
    if "nc" not in _NC_CACHE:
        _NC_CACHE["nc"] = build_nc()
    return _NC_CACHE["nc"]


def _run(prediction, coordinates, **kw):
    nc = _get_nc()
    pred = np.ascontiguousarray(np.asarray(prediction), dtype=np.float32)
    crds = np.ascontiguousarray(np.asarray(coordinates), dtype=np.int32)
    assert pred.shape == (B, 1, H, W) and crds.shape == (B, NKP, 2)
    in_maps = []
    for core in range(NCORES):
        sl = slice(core * IMGS, (core + 1) * IMGS)
        in_maps.append({
            "pred": np.ascontiguousarray(pred[sl, 0]),
            "coords": np.ascontiguousarray(crds[sl]),
        })
    res = run_bass_kernel_spmd(nc, in_maps, core_ids=list(range(NCORES)), **kw)
    s1 = s2 = s3 = 0.0
    for r in res.results:
        p = r["partial"].astype(np.float64)
        s1 += p[:, 0:IMGS * NTL].sum()
        s2 += p[:, IMGS * NTL:IMGS * NTL + IMGS].sum()
        s3 += p[:, IMGS * NTL + IMGS:].sum()
    loss = np.asarray((s1 - 2.0 * s2 + s3) / (B * H * W), dtype=np.float32)
    return loss, res


def kernel(prediction, coordinates, labels=None, gaussian_kernel=None, **kw):
    loss, _ = _run(prediction, coordinates)
    return loss


# revision 15
# speedup vs baseline: 1.4939x; 1.4939x over previous
"""Trainium2 Bass kernel for nn_LossWithBeliveMaps.

loss = mean((prediction - belive_map)^2) where belive_map is 100 Gaussian
(9x9, sigma=2) stamps per image, scattered at integer keypoint coordinates.

Decomposition (per image):  loss*N = S1 - 2*S2 + S3 with
  S1 = sum(pred^2)            -- streamed square+accumulate, no dependency
                                 on keypoints, starts as soon as DMA lands.
  S2 = sum(pred * bm)         -- bm = Ay^T Bx is rank-100 separable
                                 (G[i,j] = u(i)u(j), u(d) = exp(-d^2/8)), so
                                 S2 = sum_k w_k sum_c U[k,c] Bx[k,c] with
                                 U = Ayt^T @ pred contracted on the PE per
                                 128-row block (pred streams as float32r at
                                 full rate), then one tiny [100,1024] fused
                                 multiply-reduce on DVE.
  S3 = sum(bm^2)              -- = sum_{k,k'} w_k w_k' gy[k,k'] gx[k,k']
                                 via [100,100] Gram matmuls of the factors.
  w_k in {0,1} removes duplicate keypoints (.at[].set semantics); weights
  are applied on the small [100,*] tensors only.

The 9x9 hard cutoff of the reference kernel is approximated by the full
Gaussian tails (exp(-25/8) ~ 0.04 max excess, shifting S2/S3 by ~0.2%,
i.e. ~3e-6 relative on the loss -- tolerance is 2e-2).

Sharding: data-parallel over batch, 2 images per core, 8 cores; host sums
per-core partial columns in float64.
"""

import numpy as np

import concourse.bass as bass
import concourse.bacc as bacc
import concourse.bass_isa as bass_isa
import concourse.mybir as mybir
from concourse import tile
from concourse.bass_utils import run_bass_kernel_spmd

F32 = mybir.dt.float32
F32R = mybir.dt.float32r
I32 = mybir.dt.int32
BF16 = mybir.dt.bfloat16
OP = mybir.AluOpType
AF = mybir.ActivationFunctionType

B, H, W = 16, 1024, 1024
NKP = 100
NCORES = 8
IMGS = B // NCORES            # 2 images per core
NT = H // 128                 # 8 row-blocks per image
RB = 2                        # row-blocks per pred tile
NTL = NT // RB                # 4 pred tiles per image
# acc columns: [0..7] S1 per tile, [8..9] S2 per image, [10..11] S3
NCOL = IMGS * NTL + 2 * IMGS


def build_nc():
    nc = bacc.Bacc(None, target_bir_lowering=False)

    pred = nc.dram_tensor("pred", [IMGS, H, W], F32R, kind="ExternalInput")
    coords = nc.dram_tensor("coords", [IMGS, NKP, 2], I32, kind="ExternalInput")
    out = nc.dram_tensor("partial", [128, NCOL], F32, kind="ExternalOutput")

    with tile.TileContext(nc) as tc:
        with (
            tc.tile_pool(name="const", bufs=1) as constp,
            tc.tile_pool(name="fact", bufs=1) as factp,
            tc.tile_pool(name="pred", bufs=IMGS * (NT // RB)) as predp,
            tc.tile_pool(name="work", bufs=2) as workp,
            tc.tile_pool(name="small", bufs=2) as smallp,
            tc.tile_pool(name="acc", bufs=1) as accp,
            tc.tile_pool(name="psum", bufs=1, space="PSUM") as psump,
        ):
            acc = accp.tile([128, NCOL], F32)
            nc.gpsimd.memset(acc[:], 0)

            # ---- constants, built on-chip (no DMA bandwidth spent) ----
            iota_i = constp.tile([128, W], I32)
            nc.gpsimd.iota(iota_i[:], [[1, W]], channel_multiplier=0)
            iota_f = constp.tile([128, W], F32)
            nc.vector.tensor_copy(iota_f[:], iota_i[:])

            iotap_i = constp.tile([128, 1], I32)
            nc.gpsimd.iota(iotap_i[:], [[1, 1]], channel_multiplier=1)
            iotap_f = constp.tile([128, 1], F32)
            nc.vector.tensor_copy(iotap_f[:], iotap_i[:])

            # rconst[p, a] = p + 128*a  (row index; broadcast over k)
            rconst_f = constp.tile([128, NT], F32)
            nc.vector.tensor_scalar(rconst_f[:], iota_f[:, 0:NT], 128.0,
                                    iotap_f[:], OP.mult, OP.add)
            ones_col = constp.tile([NKP, 1], F32)
            nc.gpsimd.memset(ones_col[:], 1.0)

            # mask_lt[k, k'] = (k' < k), mask_gt[k, k'] = (k' > k)
            mask_lt = constp.tile([NKP, NKP], F32)
            nc.vector.tensor_scalar(mask_lt[:], iota_f[0:NKP, 0:NKP],
                                    iotap_f[0:NKP], None, OP.is_lt)
            mask_gt = constp.tile([NKP, NKP], F32)
            nc.vector.tensor_scalar(mask_gt[:], iota_f[0:NKP, 0:NKP],
                                    iotap_f[0:NKP], None, OP.is_gt)

            # ---- prediction layout: [128, RB, W] tiles ----
            pred_v = pred.rearrange("i (t b p) w -> i t p b w", b=RB, p=128)
            pts = {}

            def load_pt(img, t):
                pt = predp.tile([128, RB, W], F32R, tag="pt",
                                name=f"pt{img}_{t}")
                nc.sync.dma_start(pt[:], pred_v[img, t])
                pts[(img, t)] = pt

            load_pt(0, 0)
            load_pt(0, 1)
            ccs, crxs, crys = [], [], []
            for img in range(IMGS):
                cc = smallp.tile([NKP, 2], I32, tag=f"cc{img}", bufs=1)
                nc.sync.dma_start(cc[:], coords[img])
                ctv = coords[img].rearrange("n t -> t n")
                crx = smallp.tile([1, NKP], I32, tag=f"crx{img}", bufs=1)
                nc.sync.dma_start(crx[:], ctv[0:1, :])
                cry = smallp.tile([1, NKP], I32, tag=f"cry{img}", bufs=1)
                nc.sync.dma_start(cry[:], ctv[1:2, :])
                ccs.append(cc); crxs.append(crx); crys.append(cry)
            for img in range(IMGS):
                for t in range(NTL):
                    if (img, t) not in pts:
                        load_pt(img, t)

            # ---- tiny coordinate conversions for both images first ----
            ayts, bxts, bx0s = [], [], []
            xbs, ybs, ccfs = [], [], []
            for img in range(IMGS):
                ccf = smallp.tile([NKP, 2], F32, tag="ccf", bufs=1,
                                  name=f"ccf{img}")
                nc.vector.tensor_copy(ccf[:], ccs[img][:])
                crxf = smallp.tile([1, NKP], F32, tag="crxf", bufs=2,
                                   name=f"crxf{img}")
                nc.vector.tensor_copy(crxf[:], crxs[img][:])
                cryf = smallp.tile([1, NKP], F32, tag="cryf", bufs=2,
                                   name=f"cryf{img}")
                nc.vector.tensor_copy(cryf[:], crys[img][:])
                xb = smallp.tile([128, NKP], F32, tag=f"xb{img}", bufs=1)
                nc.gpsimd.partition_broadcast(xb[:], crxf[:])
                yb = smallp.tile([128, NKP], F32, tag=f"yb{img}", bufs=1)
                nc.gpsimd.partition_broadcast(yb[:], cryf[:])
                ccfs.append(ccf); xbs.append(xb); ybs.append(yb)

            # ---- factor chains: d (DVE) -> d^2 (GpSimd) -> exp (ACT) ----
            for img in range(IMGS):
                facs = []
                for bvec, dtag in ((ybs[img], "dy"), (xbs[img], "dx")):
                    bexp = bvec[:].unsqueeze(1).broadcast_to([128, NT, NKP])
                    d = workp.tile([128, NT, NKP], F32, tag="d")
                    rexp = rconst_f[:].unsqueeze(2).broadcast_to(
                        [128, NT, NKP])
                    nc.vector.tensor_tensor(d[:], rexp, bexp, OP.subtract)
                    dsq = workp.tile([128, NT, NKP], F32, tag="dsq")
                    nc.vector.tensor_tensor(dsq[:], d[:], d[:], OP.mult)
                    f = factp.tile([128, NT, NKP], F32R, tag=f"{dtag}{img}")
                    nc.scalar.activation(f[:], dsq[:], AF.Exp, scale=-0.125)
                    facs.append(f)
                ayt, bxt = facs
                ayts.append(ayt); bxts.append(bxt)

                bd = workp.tile([NKP, W], F32, tag="bd")
                nc.vector.tensor_scalar(bd[:], iota_f[0:NKP, :],
                                        ccfs[img][:, 0:1], None, OP.subtract)
                bdsq = workp.tile([NKP, W], F32, tag="bdsq")
                nc.vector.tensor_tensor(bdsq[:], bd[:], bd[:], OP.mult)
                bx0 = factp.tile([NKP, W], F32, tag=f"bx0_{img}")
                nc.scalar.activation(bx0[:], bdsq[:], AF.Exp, scale=-0.125)
                bx0s.append(bx0)

            # ---- dedup weights (off the factor critical path) ----
            wcols, wrows, walls = [], [], []
            for img in range(IMGS):
                idb = smallp.tile([NKP, NKP], F32, tag="idb")
                nc.vector.tensor_scalar(idb[:], ybs[img][0:NKP, :], 1024.0,
                                        None, OP.mult)
                nc.vector.tensor_tensor(idb[:], idb[:], xbs[img][0:NKP, :],
                                        OP.add)
                idc = smallp.tile([NKP, 1], F32, tag="idc")
                nc.vector.tensor_scalar(idc[:], ccfs[img][:, 1:2], 1024.0,
                                        ccfs[img][:, 0:1], OP.mult, OP.add)
                eq = smallp.tile([NKP, NKP], F32, tag="eq")
                nc.vector.tensor_scalar(eq[:], idb[:], idc[:], None,
                                        OP.is_equal)
                e1 = smallp.tile([NKP, NKP], F32, tag="e1")
                nc.vector.tensor_tensor(e1[:], eq[:], mask_lt[:], OP.mult)
                dup = smallp.tile([NKP, 1], F32, tag="dup")
                nc.vector.tensor_reduce(dup[:], e1[:],
                                        axis=mybir.AxisListType.X, op=OP.add)
                w_col = smallp.tile([NKP, 1], F32, tag=f"wcol{img}", bufs=1)
                nc.vector.tensor_scalar(w_col[:], dup[:], 0.0, None, OP.is_le)
                e2 = smallp.tile([NKP, NKP], F32, tag="e2")
                nc.vector.tensor_tensor(e2[:], eq[:], mask_gt[:], OP.mult)
                cntr = psump.tile([1, NKP], F32, tag=f"cntr{img}",
                                  name=f"cntr{img}")
                nc.tensor.matmul(cntr[:], ones_col[:], e2[:],
                                 start=True, stop=True)
                wrow = smallp.tile([1, NKP], F32, tag=f"wrow{img}",
                                   bufs=1, name=f"wrow{img}")
                nc.vector.tensor_scalar(wrow[:], cntr[:], 0.0, None, OP.is_le)
                wcols.append(w_col); wrows.append(wrow)

            for img in range(IMGS):
                w_all = smallp.tile([NKP, NKP], F32, tag=f"wall{img}",
                                    bufs=1, name=f"wall{img}")
                nc.gpsimd.partition_broadcast(w_all[:], wrows[img][:])
                walls.append(w_all)

            # ---- PSUM: U [100, 1024] per image + grams packed in 1 bank
            us = [psump.tile([NKP, W], F32, tag=f"u{img}", name=f"u{img}")
                  for img in range(IMGS)]
            ggs = [psump.tile([NKP, 256], F32, tag=f"gg{img}", name=f"gg{img}")
                   for img in range(IMGS)]

            # ---- Gram matmuls (bf16, tiny): gy = Ayt^T Ayt, gx = Bxt^T Bxt
            for img in range(IMGS):
                for a in range(NT):
                    ay = ayts[img][:, a, :]
                    nc.tensor.matmul(ggs[img][:, 0:NKP], ay, ay,
                                     start=(a == 0), stop=(a == NT - 1))
                for a in range(NT):
                    bx = bxts[img][:, a, :]
                    nc.tensor.matmul(ggs[img][:, 128:128 + NKP], bx, bx,
                                     start=(a == 0), stop=(a == NT - 1))

            # ---- main stream: S1 square+accum and U accumulation ----
            for img in range(IMGS):
                for t in range(NTL):
                    pt = pts[(img, t)]
                    col = img * NTL + t
                    junk = workp.tile([128, RB, W], BF16, tag="junk_act")
                    nc.scalar.activation(junk[:], pt[:].bitcast(F32),
                                         AF.Square,
                                         accum_out=acc[:, col:col + 1])
                    for b in range(RB):
                        for s in range(2):
                            nc.tensor.matmul(
                                us[img][:, s * 512:(s + 1) * 512],
                                ayts[img][:, RB * t + b, :],
                                pt[:, b, s * 512:(s + 1) * 512],
                                start=(t == 0 and b == 0),
                                stop=(t == NTL - 1 and b == RB - 1))

                # -- close out image: S2 and S3 reductions
                s2c = smallp.tile([NKP, 1], F32, tag="s2c")
                junk2 = workp.tile([NKP, W], F32, tag="junk2")
                nc.vector.tensor_tensor(junk2[:], us[img][:], bx0s[img][:],
                                        OP.mult)
                nc.vector.tensor_reduce(s2c[:], junk2[:],
                                        axis=mybir.AxisListType.X, op=OP.add)
                nc.vector.tensor_tensor(acc[0:NKP, IMGS * NTL + img:IMGS * NTL + img + 1],
                                        s2c[:], wcols[img][:], OP.mult)

                t1 = smallp.tile([NKP, NKP], F32, tag="t1")
                nc.vector.tensor_tensor(t1[:], ggs[img][:, 0:NKP],
                                        walls[img][:], OP.mult)
                s3c = smallp.tile([NKP, 1], F32, tag="s3c")
                junk3 = smallp.tile([NKP, NKP], F32, tag="junk3")
                nc.vector.tensor_tensor(junk3[:], t1[:],
                                        ggs[img][:, 128:128 + NKP], OP.mult)
                nc.vector.tensor_reduce(s3c[:], junk3[:],
                                        axis=mybir.AxisListType.X, op=OP.add)
                nc.vector.tensor_tensor(acc[0:NKP, IMGS * NTL + IMGS + img:IMGS * NTL + IMGS + img + 1],
                                        s3c[:], wcols[img][:], OP.mult)

            nc.sync.dma_start(out[:], acc[:])

    nc.compile()
    return nc


_NC_CACHE = {}


def _get_nc():
    if "nc" not in _NC_CACHE:
        _NC_CACHE["nc"] = build_nc()
    return _NC_CACHE["nc"]


def _run(prediction, coordinates, **kw):
    nc = _get_nc()
    pred = np.ascontiguousarray(np.asarray(prediction), dtype=np.float32)
    crds = np.ascontiguousarray(np.asarray(coordinates), dtype=np.int32)
    assert pred.shape == (B, 1, H, W) and crds.shape == (B, NKP, 2)
    in_maps = []
    for core in range(NCORES):
        sl = slice(core * IMGS, (core + 1) * IMGS)
        in_maps.append({
            "pred": np.ascontiguousarray(pred[sl, 0]),
            "coords": np.ascontiguousarray(crds[sl]),
        })
    res = run_bass_kernel_spmd(nc, in_maps, core_ids=list(range(NCORES)), **kw)
    s1 = s2 = s3 = 0.0
    for r in res.results:
        p = r["partial"].astype(np.float64)
        s1 += p[:, 0:IMGS * NTL].sum()
        s2 += p[:, IMGS * NTL:IMGS * NTL + IMGS].sum()
        s3 += p[:, IMGS * NTL + IMGS:].sum()
    loss = np.asarray((s1 - 2.0 * s2 + s3) / (B * H * W), dtype=np.float32)
    return loss, res


def kernel(prediction, coordinates, labels=None, gaussian_kernel=None, **kw):
    loss, _ = _run(prediction, coordinates)
    return loss
